# revision 23
# baseline (speedup 1.0000x reference)
"""LayerNorm-LSTMCell Bass kernel for Trainium2, data-parallel over batch on 8 NeuronCores.

Computes, per the reference nn.Module:
    gates = x @ W_i + h_prev @ W_h + b          # [B, 4H], gate order i|f|g|o
    i, f, g, o = split(gates);  i,f,o = sigmoid; g = tanh
    c = f * c_prev + i * g
    h = LayerNorm(o * tanh(c)) * ln_weight + ln_bias
Returns (h, c), both [B, H] fp32.

Sharding: batch B=16384 split 8 ways (2048 rows/core); weights replicated.
Each core's x / h_prev shard is staged feature-major (transposed on host as
part of sharding) so the tensor engine can use it directly as the stationary
matmul operand; c_prev and all outputs stay batch-major.

Per-core design notes (v5):
  - Matmuls in bf16 (fp32 is 4x slower on the PE; fp8 DoubleRow fails the
    2e-2 accuracy gate: measured 3.4e-2 end-to-end), fp32 PSUM accumulation.
  - xT / hT / c_prev are downcast to bf16 by SWDGE cast-DMA loads; W is
    cast-loaded bf16 in 8 gate-column DMAs so the first gate's matmuls can
    start as soon as one-eighth of W has landed. The first quad of batch
    tiles is processed gate-major so the PE chases the W stream without
    stalling; later quads run tile-major for epilogue locality.
  - Gates accumulate chunk-wise: one 512-col PSUM bank per gate, 8 K-block
    matmuls each; the scalar engine drains each bank with one activation
    (sigmoid/tanh) with the gate's bias folded in as an immediate when b is
    per-gate constant (checked at build time from the actual b; otherwise a
    broadcast bias tile is added on the vector engine).
  - Epilogue largely in bf16 so DVE runs in 2x/4x perf modes; c stays fp32.
  - LayerNorm stats via bn_stats/bn_aggr; 1/sqrt(var+eps) by 2 Newton
    iterations (int32 bit-trick seed) on the vector engine, batched 4 tiles
    at a time except a 2/1/1 split at the end to shorten the tail; the last
    quad stores c/h per-tile for the same reason. ln_weight/ln_bias
    application is skipped when they are identity (checked at build time),
    else applied on the idle GPSIMD engine.
"""

import numpy as np

N_CORES = 8
B, I_DIM, H = 16384, 512, 512
G4 = 4 * H  # 2048
BS = B // N_CORES  # 2048 batch rows per core
P = 128
NT = BS // P  # 16 batch tiles per core
QUAD = 4  # batch tiles batched per DMA instruction
LN_GROUPS = [4, 4, 4, 2, 1, 1]  # tiles per rsqrt batch (short tail)
NEWTON_ITERS = 2
LN_EPS = 1e-5
RSQRT_MAGIC = 0x5F3759DF
LOAD_BUFS = 3
GSB_BUFS = 3
PSUM_G_BUFS = 7
N_WARMUP_MM = 7   # dummy matmuls that absorb the PE p-state ramp during load
N_WARMUP_MM2 = 0   # filler matmuls bridging the W_h-gate-0 arrival

_CACHE = {}


def _emit(nc, tc, ctx, gate_bias, ln_identity):
    import concourse.bass as bass
    import concourse.mybir as mybir

    F32, BF16, I32 = mybir.dt.float32, mybir.dt.bfloat16, mybir.dt.int32
    AF = mybir.ActivationFunctionType
    OP = mybir.AluOpType

    # x / h_prev arrive feature-major (transposed per-shard on host) and all
    # matmul operands plus c_prev arrive pre-cast to bf16, so every load is a
    # cast-free HWDGE DMA (no SWDGE descriptor-generation serialization).
    xt_d = nc.dram_tensor("x", [I_DIM, BS], BF16, kind="ExternalInput").ap()
    ht_d = nc.dram_tensor("h_prev", [H, BS], BF16, kind="ExternalInput").ap()
    c_d = nc.dram_tensor("c_prev", [BS, H], BF16, kind="ExternalInput").ap()
    wi_d = nc.dram_tensor("W_i", [I_DIM, G4], BF16, kind="ExternalInput").ap()
    wh_d = nc.dram_tensor("W_h", [H, G4], BF16, kind="ExternalInput").ap()
    b_d = nc.dram_tensor("b", [G4], F32, kind="ExternalInput").ap()
    lnw_d = nc.dram_tensor("ln_weight", [H], F32, kind="ExternalInput").ap()
    lnb_d = nc.dram_tensor("ln_bias", [H], F32, kind="ExternalInput").ap()
    # outputs stored bf16 (exact f32 upcast on host; well within tolerance)
    ho_d = nc.dram_tensor("h_out", [BS, H], BF16, kind="ExternalOutput").ap()
    co_d = nc.dram_tensor("c_out", [BS, H], BF16, kind="ExternalOutput").ap()

    KX = I_DIM // P  # 4 k-blocks from x
    KH = H // P      # 4 k-blocks from h_prev
    KK = KX + KH     # 8

    consts = ctx.enter_context(tc.tile_pool(name="consts", bufs=1))
    loads = ctx.enter_context(tc.tile_pool(name="loads", bufs=LOAD_BUFS))
    gsb_pool = ctx.enter_context(tc.tile_pool(name="gsb", bufs=GSB_BUFS))
    epi = ctx.enter_context(tc.tile_pool(name="epi", bufs=3))
    outq = ctx.enter_context(tc.tile_pool(name="outq", bufs=2))
    hpre_pool = ctx.enter_context(tc.tile_pool(name="hpre", bufs=QUAD + 2))
    stat_pool = ctx.enter_context(tc.tile_pool(name="stats", bufs=3))
    grp_pool = ctx.enter_context(tc.tile_pool(name="grp", bufs=2))
    psum_g = ctx.enter_context(tc.tile_pool(name="psum_g", bufs=PSUM_G_BUFS, space="PSUM"))
    psum_w = ctx.enter_context(tc.tile_pool(name="psum_w", bufs=1, space="PSUM"))

    magic = consts.tile([P, QUAD], I32)
    nc.vector.memset(magic, RSQRT_MAGIC)

    # Dummy matmuls keep the PE continuously busy (and its p-state ramp warm)
    # while the first activation/weight DMAs stream in; results are discarded.
    warm_sb = consts.tile([P, P + H], BF16)
    nc.vector.memset(warm_sb, 0.0)
    warm_lhs, warm_rhs = warm_sb[:, 0:P], warm_sb[:, P:P + H]
    warm_ps = psum_w.tile([P, H], F32)

    def warm_mms(n):
        for _ in range(n):
            nc.tensor.matmul(warm_ps[:], warm_lhs, warm_rhs,
                             start=True, stop=True)

    warm_mms(N_WARMUP_MM)

    # Gate activation schedule: index = gate slot in i|f|g|o order.
    gate_funcs = [AF.Sigmoid, AF.Sigmoid, AF.Tanh, AF.Sigmoid]

    def dram_quad(ap2d, q):
        return ap2d[q * QUAD * P:(q + 1) * QUAD * P, :].rearrange(
            "(n p) d -> p n d", p=P)

    # xh_T[p, k, col]: feature-major activations, k-blocks 0..3 from x,
    # 4..7 from h_prev; col = batch index within the shard.
    xh_T = consts.tile([P, KK, BS], BF16)
    quad_c = {}
    out_tiles = {}

    def load_quad_xh(q):
        cols = slice(q * QUAD * P, (q + 1) * QUAD * P)
        for base, src in ((0, xt_d), (KX, ht_d)):
            rows = src[:, cols].rearrange("(k p) n -> p k n", p=P)
            nc.sync.dma_start(out=xh_T[:, base:base + KX, cols], in_=rows)
        c4_sb = outq.tile([P, QUAD, H], BF16, tag="c4_sb")
        h4_sb = outq.tile([P, QUAD, H], BF16, tag="h4_sb")
        out_tiles[q] = (c4_sb, h4_sb)

    def load_quad_c(q):
        c4 = loads.tile([P, QUAD, H], BF16, tag="c4")
        nc.sync.dma_start(out=c4[:], in_=dram_quad(c_d, q))
        quad_c[q] = c4

    # --- W load: one DMA per (source tensor, gate column block) --------------
    # w_all[p, k, g] = [W_i; W_h] row 128k+p, col g  (k-major bf16 layout).
    w_all = consts.tile([P, KK, G4], BF16)

    def load_w_gate(gate):
        cols = slice(gate * H, (gate + 1) * H)
        for half, src in ((0, wi_d), (1, wh_d)):
            rows = src[:, cols].rearrange("(k p) g -> p k g", p=P)
            nc.sync.dma_start(
                out=w_all[:, half * KX:(half + 1) * KX, cols], in_=rows)

    if gate_bias is None:
        # General path: bias varies within a gate; broadcast to all partitions
        # and add into PSUM on the vector engine before activations.
        b_bc = consts.tile([P, G4], F32)
        b_src = bass.AP(tensor=b_d.tensor, offset=b_d.offset,
                        ap=[[0, P], [1, G4]])
        nc.sync.dma_start(out=b_bc[:], in_=b_src)

    if not ln_identity:
        lnw_bc = bass.AP(tensor=lnw_d.tensor, offset=lnw_d.offset,
                         ap=[[0, P]] + [list(a) for a in lnw_d.ap])
        lnw_b = consts.tile([P, H], F32)
        nc.sync.dma_start(out=lnw_b[:], in_=lnw_bc)
        lnb_bc = bass.AP(tensor=lnb_d.tensor, offset=lnb_d.offset,
                         ap=[[0, P]] + [list(a) for a in lnb_d.ap])
        lnb_b = consts.tile([P, H], F32)
        nc.sync.dma_start(out=lnb_b[:], in_=lnb_bc)

    # Startup DMA order: x / W_i gate-0 in k-halves (unblocking the first
    # k0-1 matmuls as early as possible), then h / W_h gate 0, then the rest.
    cols0 = slice(0, QUAD * P)
    gcols0 = slice(0, H)
    for klo, khi in ((0, 2), (2, 4)):
        rsl = slice(klo * P, khi * P)
        nc.sync.dma_start(
            out=xh_T[:, klo:khi, cols0],
            in_=xt_d[rsl, cols0].rearrange("(k p) n -> p k n", p=P))
        nc.sync.dma_start(
            out=w_all[:, klo:khi, gcols0],
            in_=wi_d[rsl, gcols0].rearrange("(k p) g -> p k g", p=P))
    nc.sync.dma_start(out=xh_T[:, KX:KK, cols0],
                        in_=ht_d[:, cols0].rearrange("(k p) n -> p k n", p=P))
    nc.sync.dma_start(out=w_all[:, KX:KK, gcols0],
                        in_=wh_d[:, gcols0].rearrange("(k p) g -> p k g", p=P))
    c4_sb0 = outq.tile([P, QUAD, H], BF16, tag="c4_sb")
    h4_sb0 = outq.tile([P, QUAD, H], BF16, tag="h4_sb")
    out_tiles[0] = (c4_sb0, h4_sb0)
    load_w_gate(1)
    load_quad_c(0)
    load_w_gate(2)
    load_w_gate(3)
    load_quad_xh(1)
    load_quad_c(1)

    # --- per-tile pieces -----------------------------------------------------
    gsbs = {}

    psum_c = {}  # (t, gate) -> partially-accumulated PSUM chunk

    def mm_gate_ks(t, gate, k_lo, k_hi):
        if (t, gate) in psum_c:
            Gc = psum_c[(t, gate)]
        else:
            Gc = psum_c[(t, gate)] = psum_g.tile([P, H], F32, name="Gc", tag="Gc")
        cols = slice(gate * H, (gate + 1) * H)
        bcols = slice(t * P, (t + 1) * P)
        for k in range(k_lo, k_hi):
            nc.tensor.matmul(Gc[:], xh_T[:, k, bcols], w_all[:, k, cols],
                             start=(k == 0), stop=(k == KK - 1))
        if k_hi < KK:
            return
        del psum_c[(t, gate)]
        if t not in gsbs:
            gsbs[t] = gsb_pool.tile([P, 4, H], BF16, name="gsb", tag="gsb")
        if gate_bias is None:
            nc.vector.tensor_add(Gc[:], Gc[:], b_bc[:, cols])
            bias_imm = 0.0
        else:
            bias_imm = float(gate_bias[gate])
        nc.scalar.activation(gsbs[t][:, gate, :], Gc[:], gate_funcs[gate],
                             bias=bias_imm)

    def mm_gate(t, gate):
        mm_gate_ks(t, gate, 0, KK)

    tanh_cs = {}

    def epilogue_c_phase(t):
        # c = f*c_prev + i*g, tanh(c), and the c store
        q, tq = divmod(t, QUAD)
        c4 = quad_c[q]
        c4_sb, _ = out_tiles[q]
        gsb = gsbs[t]
        i_s, f_s, g_t = gsb[:, 0, :], gsb[:, 1, :], gsb[:, 2, :]
        tmp = epi.tile([P, H], BF16, tag="tmp")
        nc.vector.tensor_mul(tmp[:], i_s, g_t)
        c1 = epi.tile([P, H], BF16, tag="c1")
        nc.gpsimd.tensor_mul(c1[:], f_s, c4[:, tq, :])
        nc.vector.tensor_add(c4_sb[:, tq, :], c1[:], tmp[:])
        tanh_c = epi.tile([P, H], BF16, tag="tanh_c")
        nc.scalar.activation(tanh_c[:], c4_sb[:, tq, :], AF.Tanh)
        tanh_cs[t] = tanh_c
        if q == NT // QUAD - 1:  # per-tile store in the last quad (short tail)
            rows = slice(t * P, (t + 1) * P)
            nc.sync.dma_start(out=co_d[rows, :].rearrange("(n p) d -> p n d", p=P),
                              in_=c4_sb[:, tq:tq + 1, :])
        elif tq == QUAD - 1:
            nc.sync.dma_start(out=dram_quad(co_d, q), in_=c4_sb[:])

    def epilogue_tile(t, mv_g, tt, hpres):
        if t not in tanh_cs:
            epilogue_c_phase(t)
        tanh_c = tanh_cs.pop(t)
        o_s = gsbs.pop(t)[:, 3, :]
        h_pre = hpre_pool.tile([P, H], BF16, tag="h_pre")
        nc.vector.tensor_mul(h_pre[:], o_s, tanh_c[:])
        st = stat_pool.tile([P, 6], F32, tag="st")
        nc.vector.bn_stats(out=st[:], in_=h_pre[:])
        nc.vector.bn_aggr(out=mv_g[:, tt, :], in_=st[:])
        hpres.append((t, h_pre))

    def ln_group(g_tiles, mv_g, hpres):
        g_sz = len(g_tiles)
        # ---- batched 1/sqrt(var+eps) via Newton (vector engine only) -------
        mu_v = mv_g[:, 0:g_sz, 0]
        var_v = mv_g[:, 0:g_sz, 1]
        inv_g = grp_pool.tile([P, QUAD], F32, tag="inv_g")
        nms_g = grp_pool.tile([P, QUAD], F32, tag="nms_g")
        v_g = grp_pool.tile([P, QUAD], F32, tag="v_g")
        nc.vector.tensor_scalar_add(v_g[:, 0:g_sz], var_v, LN_EPS)
        y_i = inv_g.bitcast(I32)
        nc.vector.tensor_scalar(y_i[:, 0:g_sz], v_g[:, 0:g_sz].bitcast(I32),
                                1, None, op0=OP.logical_shift_right)
        nc.vector.tensor_sub(y_i[:, 0:g_sz], magic[:, 0:g_sz], y_i[:, 0:g_sz])
        nt1 = grp_pool.tile([P, QUAD], F32, tag="nt1")
        # 1 iteration for the 1-wide tail groups: inv rel-err ~1.7e-3 over
        # 1/16 of rows -> ~4e-4 on h; shortens the critical tail chain.
        iters = 1 if g_sz == 1 else NEWTON_ITERS
        for _ in range(iters):  # Newton: y = y * (1.5 - 0.5 * v * y^2)
            nc.vector.tensor_mul(nt1[:, 0:g_sz], inv_g[:, 0:g_sz], inv_g[:, 0:g_sz])
            nc.vector.tensor_mul(nt1[:, 0:g_sz], nt1[:, 0:g_sz], v_g[:, 0:g_sz])
            nc.vector.tensor_scalar(nt1[:, 0:g_sz], nt1[:, 0:g_sz], -0.5, 1.5,
                                    op0=OP.mult, op1=OP.add)
            nc.vector.tensor_mul(inv_g[:, 0:g_sz], inv_g[:, 0:g_sz], nt1[:, 0:g_sz])
        nc.vector.scalar_tensor_tensor(nms_g[:, 0:g_sz], mu_v, -1.0,
                                       inv_g[:, 0:g_sz], op0=OP.mult, op1=OP.mult)

        # ---- normalize (+ optional ln scale/shift) + store ------------------
        for tt, (t, h_pre) in enumerate(hpres):
            q, tq = divmod(t, QUAD)
            c4_sb, h4_sb = out_tiles[q]
            if ln_identity and t == NT - 1:
                # last tile: normalize on DVE right after the Newton chain
                # (no cross-engine hop before the final store)
                nc.vector.tensor_scalar(h4_sb[:, tq, :], h_pre[:],
                                        inv_g[:, tt:tt + 1],
                                        nms_g[:, tt:tt + 1],
                                        op0=OP.mult, op1=OP.add)
                rows = slice(t * P, (t + 1) * P)
                nc.sync.dma_start(
                    out=ho_d[rows, :].rearrange("(n p) d -> p n d", p=P),
                    in_=h4_sb[:, tq:tq + 1, :])
                continue
            if ln_identity:
                nc.scalar.activation(h4_sb[:, tq, :], h_pre[:], AF.Identity,
                                     bias=nms_g[:, tt:tt + 1],
                                     scale=inv_g[:, tt:tt + 1])
            else:
                h_n = epi.tile([P, H], F32, tag="h_n")
                nc.scalar.activation(h_n[:], h_pre[:], AF.Identity,
                                     bias=nms_g[:, tt:tt + 1],
                                     scale=inv_g[:, tt:tt + 1])
                h1 = epi.tile([P, H], F32, tag="h1")
                nc.gpsimd.tensor_mul(h1[:], h_n[:], lnw_b[:])
                nc.gpsimd.tensor_add(h4_sb[:, tq, :], h1[:], lnb_b[:])
            if q == NT // QUAD - 1:
                rows = slice(t * P, (t + 1) * P)
                nc.sync.dma_start(
                    out=ho_d[rows, :].rearrange("(n p) d -> p n d", p=P),
                    in_=h4_sb[:, tq:tq + 1, :])
            elif tq == QUAD - 1:
                nc.sync.dma_start(out=dram_quad(ho_d, q), in_=h4_sb[:])

    # --- main schedule -------------------------------------------------------
    # Quad 0 runs gate-major so the PE chases the 8 streaming W DMAs without
    # stalling; later quads run tile-major.
    group_of_tile = {}
    groups = []
    t0 = 0
    for sz in LN_GROUPS:
        groups.append(list(range(t0, t0 + sz)))
        for t in range(t0, t0 + sz):
            group_of_tile[t] = len(groups) - 1
        t0 += sz
    group_state = {}  # group idx -> (mv_g, hpres)

    def finish_tile(t):
        gi = group_of_tile[t]
        if gi not in group_state:
            mv_g = grp_pool.tile([P, QUAD, 2], F32, name="mv_g", tag="mv_g")
            group_state[gi] = (mv_g, [])
        mv_g, hpres = group_state[gi]
        tt = t - groups[gi][0]
        epilogue_tile(t, mv_g, tt, hpres)
        if t == groups[gi][-1]:
            ln_group(groups[gi], mv_g, hpres)

    # quad 0 (gate-major; gate 0 additionally split by k-phase so the PE can
    # start on the first x / W_i k-halves while the rest is still streaming;
    # small warm-up filler blocks bridge load-arrival boundaries)
    for t in range(QUAD):
        mm_gate_ks(t, 0, 0, 2)
    for t in range(QUAD):
        mm_gate_ks(t, 0, 2, KX)
    warm_mms(N_WARMUP_MM2)
    for t in range(QUAD):
        mm_gate_ks(t, 0, KX, KK)
    for gate in range(1, 4):
        for t in range(QUAD):
            mm_gate(t, gate)
    for t in range(QUAD):
        finish_tile(t)

    # quads 1..3 (tile-major). The last tile runs its gates g,i,f,o with the
    # c/tanh chain emitted before the o matmuls, so after the final matmul
    # only o -> h_pre -> stats -> rsqrt -> normalize -> store remain.
    for t in range(QUAD, NT):
        q, tq = divmod(t, QUAD)
        if tq == 0 and q + 1 < NT // QUAD:
            load_quad_xh(q + 1)
            load_quad_c(q + 1)
        if t == NT - 1:
            for gate in (2, 0, 1):
                mm_gate(t, gate)
            epilogue_c_phase(t)
            mm_gate(t, 3)
        else:
            for gate in range(4):
                mm_gate(t, gate)
        finish_tile(t)


def _build(gate_bias, ln_identity):
    key = ("nc", gate_bias, ln_identity)
    if key in _CACHE:
        return _CACHE[key]
    from contextlib import ExitStack
    import concourse.tile as tile
    from concourse import bacc

    nc = bacc.Bacc("TRN2", target_bir_lowering=False, debug=False)
    with tile.TileContext(nc) as tc:
        with ExitStack() as ctx:
            _emit(nc, tc, ctx, gate_bias, ln_identity)
    nc.compile()
    _CACHE[key] = nc
    return nc


def kernel(x, h_prev, c_prev, W_i, W_h, b, ln_weight, ln_bias):
    from concourse.bass_utils import run_bass_kernel_spmd

    b = np.asarray(b, dtype=np.float32)
    ln_weight = np.asarray(ln_weight, dtype=np.float32)
    ln_bias = np.asarray(ln_bias, dtype=np.float32)

    # Specialize the compiled program to the actual bias / LN parameters when
    # they have the common structure (per-gate-constant bias, identity LN);
    # general fallback paths otherwise.
    bg = b.reshape(4, H)
    if np.all(bg == bg[:, :1]):
        gate_bias = tuple(float(v) for v in bg[:, 0])
    else:
        gate_bias = None
    ln_identity = bool(np.all(ln_weight == 1.0) and np.all(ln_bias == 0.0))

    nc = _build(gate_bias, ln_identity)
    import ml_dtypes
    bf16 = ml_dtypes.bfloat16
    wi_b = np.asarray(W_i, dtype=bf16)
    wh_b = np.asarray(W_h, dtype=bf16)
    x = np.asarray(x)
    h_prev = np.asarray(h_prev)
    in_maps = []
    for c in range(N_CORES):
        rows = slice(c * BS, (c + 1) * BS)
        in_maps.append({
            # per-shard feature-major bf16 staging of the activations
            "x": np.ascontiguousarray(x[rows].T.astype(bf16)),
            "h_prev": np.ascontiguousarray(h_prev[rows].T.astype(bf16)),
            "c_prev": np.ascontiguousarray(c_prev[rows]).astype(bf16),
            "W_i": wi_b,
            "W_h": wh_b,
            "b": b,
            "ln_weight": ln_weight,
            "ln_bias": ln_bias,
        })
    res = run_bass_kernel_spmd(nc, in_maps, list(range(N_CORES)))
    h = np.concatenate([np.asarray(res.results[c]["h_out"], dtype=np.float32)
                        for c in range(N_CORES)], axis=0)
    c_out = np.concatenate([np.asarray(res.results[c]["c_out"], dtype=np.float32)
                            for c in range(N_CORES)], axis=0)
    return h, c_out


# revision 24
# speedup vs baseline: 1.0067x; 1.0067x over previous
"""LayerNorm-LSTMCell Bass kernel for Trainium2, data-parallel over batch on 8 NeuronCores.

Computes, per the reference nn.Module:
    gates = x @ W_i + h_prev @ W_h + b          # [B, 4H], gate order i|f|g|o
    i, f, g, o = split(gates);  i,f,o = sigmoid; g = tanh
    c = f * c_prev + i * g
    h = LayerNorm(o * tanh(c)) * ln_weight + ln_bias
Returns (h, c), both [B, H] fp32.

Sharding: batch B=16384 split 8 ways (2048 rows/core); weights replicated.
Each core's x / h_prev shard is staged feature-major (transposed on host as
part of sharding) so the tensor engine can use it directly as the stationary
matmul operand; c_prev and all outputs stay batch-major.

Per-core design notes (v5):
  - Matmuls in bf16 (fp32 is 4x slower on the PE; fp8 DoubleRow fails the
    2e-2 accuracy gate: measured 3.4e-2 end-to-end), fp32 PSUM accumulation.
  - xT / hT / c_prev are downcast to bf16 by SWDGE cast-DMA loads; W is
    cast-loaded bf16 in 8 gate-column DMAs so the first gate's matmuls can
    start as soon as one-eighth of W has landed. The first quad of batch
    tiles is processed gate-major so the PE chases the W stream without
    stalling; later quads run tile-major for epilogue locality.
  - Gates accumulate chunk-wise: one 512-col PSUM bank per gate, 8 K-block
    matmuls each; the scalar engine drains each bank with one activation
    (sigmoid/tanh) with the gate's bias folded in as an immediate when b is
    per-gate constant (checked at build time from the actual b; otherwise a
    broadcast bias tile is added on the vector engine).
  - Epilogue largely in bf16 so DVE runs in 2x/4x perf modes; c stays fp32.
  - LayerNorm stats via bn_stats/bn_aggr; 1/sqrt(var+eps) by 2 Newton
    iterations (int32 bit-trick seed) on the vector engine, batched 4 tiles
    at a time except a 2/1/1 split at the end to shorten the tail; the last
    quad stores c/h per-tile for the same reason. ln_weight/ln_bias
    application is skipped when they are identity (checked at build time),
    else applied on the idle GPSIMD engine.
"""

import numpy as np

N_CORES = 8
B, I_DIM, H = 16384, 512, 512
G4 = 4 * H  # 2048
BS = B // N_CORES  # 2048 batch rows per core
P = 128
NT = BS // P  # 16 batch tiles per core
QUAD = 4  # batch tiles batched per DMA instruction
LN_GROUPS = [4, 4, 4, 2, 1, 1]  # tiles per rsqrt batch (short tail)
NEWTON_ITERS = 2
LN_EPS = 1e-5
RSQRT_MAGIC = 0x5F3759DF
LOAD_BUFS = 3
GSB_BUFS = 3
PSUM_G_BUFS = 7
N_WARMUP_MM = 7   # dummy matmuls that absorb the PE p-state ramp during load
N_WARMUP_MM2 = 0   # filler matmuls bridging the W_h-gate-0 arrival

_CACHE = {}


def _emit(nc, tc, ctx, gate_bias, ln_identity):
    import concourse.bass as bass
    import concourse.mybir as mybir

    F32, BF16, I32 = mybir.dt.float32, mybir.dt.bfloat16, mybir.dt.int32
    AF = mybir.ActivationFunctionType
    OP = mybir.AluOpType

    # x / h_prev arrive feature-major (transposed per-shard on host) and all
    # matmul operands plus c_prev arrive pre-cast to bf16, so every load is a
    # cast-free HWDGE DMA (no SWDGE descriptor-generation serialization).
    xt_d = nc.dram_tensor("x", [I_DIM, BS], BF16, kind="ExternalInput").ap()
    ht_d = nc.dram_tensor("h_prev", [H, BS], BF16, kind="ExternalInput").ap()
    c_d = nc.dram_tensor("c_prev", [BS, H], BF16, kind="ExternalInput").ap()
    wi_d = nc.dram_tensor("W_i", [I_DIM, G4], BF16, kind="ExternalInput").ap()
    wh_d = nc.dram_tensor("W_h", [H, G4], BF16, kind="ExternalInput").ap()
    b_d = nc.dram_tensor("b", [G4], F32, kind="ExternalInput").ap()
    lnw_d = nc.dram_tensor("ln_weight", [H], F32, kind="ExternalInput").ap()
    lnb_d = nc.dram_tensor("ln_bias", [H], F32, kind="ExternalInput").ap()
    # outputs stored bf16 (exact f32 upcast on host; well within tolerance)
    ho_d = nc.dram_tensor("h_out", [BS, H], BF16, kind="ExternalOutput").ap()
    co_d = nc.dram_tensor("c_out", [BS, H], BF16, kind="ExternalOutput").ap()

    KX = I_DIM // P  # 4 k-blocks from x
    KH = H // P      # 4 k-blocks from h_prev
    KK = KX + KH     # 8

    consts = ctx.enter_context(tc.tile_pool(name="consts", bufs=1))
    loads = ctx.enter_context(tc.tile_pool(name="loads", bufs=LOAD_BUFS))
    gsb_pool = ctx.enter_context(tc.tile_pool(name="gsb", bufs=GSB_BUFS))
    epi = ctx.enter_context(tc.tile_pool(name="epi", bufs=3))
    outq = ctx.enter_context(tc.tile_pool(name="outq", bufs=2))
    hpre_pool = ctx.enter_context(tc.tile_pool(name="hpre", bufs=QUAD + 2))
    stat_pool = ctx.enter_context(tc.tile_pool(name="stats", bufs=3))
    grp_pool = ctx.enter_context(tc.tile_pool(name="grp", bufs=2))
    psum_g = ctx.enter_context(tc.tile_pool(name="psum_g", bufs=PSUM_G_BUFS, space="PSUM"))
    psum_w = ctx.enter_context(tc.tile_pool(name="psum_w", bufs=1, space="PSUM"))

    magic = consts.tile([P, QUAD], I32)
    nc.vector.memset(magic, RSQRT_MAGIC)

    # Dummy matmuls keep the PE continuously busy (and its p-state ramp warm)
    # while the first activation/weight DMAs stream in; results are discarded.
    warm_sb = consts.tile([P, P + H], BF16)
    nc.vector.memset(warm_sb, 0.0)
    warm_lhs, warm_rhs = warm_sb[:, 0:P], warm_sb[:, P:P + H]
    warm_ps = psum_w.tile([P, H], F32)

    def warm_mms(n):
        for _ in range(n):
            nc.tensor.matmul(warm_ps[:], warm_lhs, warm_rhs,
                             start=True, stop=True)

    warm_mms(N_WARMUP_MM)

    # Gate activation schedule: index = gate slot in i|f|g|o order.
    gate_funcs = [AF.Sigmoid, AF.Sigmoid, AF.Tanh, AF.Sigmoid]

    def dram_quad(ap2d, q):
        return ap2d[q * QUAD * P:(q + 1) * QUAD * P, :].rearrange(
            "(n p) d -> p n d", p=P)

    # xh_T[p, k, col]: feature-major activations, k-blocks 0..3 from x,
    # 4..7 from h_prev; col = batch index within the shard.
    xh_T = consts.tile([P, KK, BS], BF16)
    quad_c = {}
    out_tiles = {}

    def load_quad_xh(q):
        cols = slice(q * QUAD * P, (q + 1) * QUAD * P)
        for base, src in ((0, xt_d), (KX, ht_d)):
            rows = src[:, cols].rearrange("(k p) n -> p k n", p=P)
            nc.sync.dma_start(out=xh_T[:, base:base + KX, cols], in_=rows)
        c4_sb = outq.tile([P, QUAD, H], BF16, tag="c4_sb")
        h4_sb = outq.tile([P, QUAD, H], BF16, tag="h4_sb")
        out_tiles[q] = (c4_sb, h4_sb)

    def load_quad_c(q):
        c4 = loads.tile([P, QUAD, H], BF16, tag="c4")
        nc.sync.dma_start(out=c4[:], in_=dram_quad(c_d, q))
        quad_c[q] = c4

    # --- W load: one DMA per (source tensor, gate column block) --------------
    # w_all[p, k, g] = [W_i; W_h] row 128k+p, col g  (k-major bf16 layout).
    w_all = consts.tile([P, KK, G4], BF16)

    def load_w_gate(gate):
        cols = slice(gate * H, (gate + 1) * H)
        for half, src in ((0, wi_d), (1, wh_d)):
            rows = src[:, cols].rearrange("(k p) g -> p k g", p=P)
            nc.sync.dma_start(
                out=w_all[:, half * KX:(half + 1) * KX, cols], in_=rows)

    if gate_bias is None:
        # General path: bias varies within a gate; broadcast to all partitions
        # and add into PSUM on the vector engine before activations.
        b_bc = consts.tile([P, G4], F32)
        b_src = bass.AP(tensor=b_d.tensor, offset=b_d.offset,
                        ap=[[0, P], [1, G4]])
        nc.sync.dma_start(out=b_bc[:], in_=b_src)

    if not ln_identity:
        lnw_bc = bass.AP(tensor=lnw_d.tensor, offset=lnw_d.offset,
                         ap=[[0, P]] + [list(a) for a in lnw_d.ap])
        lnw_b = consts.tile([P, H], F32)
        nc.sync.dma_start(out=lnw_b[:], in_=lnw_bc)
        lnb_bc = bass.AP(tensor=lnb_d.tensor, offset=lnb_d.offset,
                         ap=[[0, P]] + [list(a) for a in lnb_d.ap])
        lnb_b = consts.tile([P, H], F32)
        nc.sync.dma_start(out=lnb_b[:], in_=lnb_bc)

    # Startup DMA order: x / W_i gate-0 in k-halves (unblocking the first
    # k0-1 matmuls as early as possible), then h / W_h gate 0, then the rest.
    cols0 = slice(0, QUAD * P)
    gcols0 = slice(0, H)
    for klo, khi in ((0, 2), (2, 4)):
        rsl = slice(klo * P, khi * P)
        nc.sync.dma_start(
            out=xh_T[:, klo:khi, cols0],
            in_=xt_d[rsl, cols0].rearrange("(k p) n -> p k n", p=P))
        nc.sync.dma_start(
            out=w_all[:, klo:khi, gcols0],
            in_=wi_d[rsl, gcols0].rearrange("(k p) g -> p k g", p=P))
    nc.sync.dma_start(out=xh_T[:, KX:KK, cols0],
                        in_=ht_d[:, cols0].rearrange("(k p) n -> p k n", p=P))
    nc.sync.dma_start(out=w_all[:, KX:KK, gcols0],
                        in_=wh_d[:, gcols0].rearrange("(k p) g -> p k g", p=P))
    c4_sb0 = outq.tile([P, QUAD, H], BF16, tag="c4_sb")
    h4_sb0 = outq.tile([P, QUAD, H], BF16, tag="h4_sb")
    out_tiles[0] = (c4_sb0, h4_sb0)
    load_w_gate(1)
    load_quad_c(0)
    load_w_gate(2)
    load_w_gate(3)
    load_quad_xh(1)
    load_quad_c(1)

    # --- per-tile pieces -----------------------------------------------------
    gsbs = {}

    psum_c = {}  # (t, gate) -> partially-accumulated PSUM chunk

    def mm_gate_ks(t, gate, k_lo, k_hi):
        if (t, gate) in psum_c:
            Gc = psum_c[(t, gate)]
        else:
            Gc = psum_c[(t, gate)] = psum_g.tile([P, H], F32, name="Gc", tag="Gc")
        cols = slice(gate * H, (gate + 1) * H)
        bcols = slice(t * P, (t + 1) * P)
        for k in range(k_lo, k_hi):
            nc.tensor.matmul(Gc[:], xh_T[:, k, bcols], w_all[:, k, cols],
                             start=(k == 0), stop=(k == KK - 1))
        if k_hi < KK:
            return
        del psum_c[(t, gate)]
        if t not in gsbs:
            gsbs[t] = gsb_pool.tile([P, 4, H], BF16, name="gsb", tag="gsb")
        if gate_bias is None:
            nc.vector.tensor_add(Gc[:], Gc[:], b_bc[:, cols])
            bias_imm = 0.0
        else:
            bias_imm = float(gate_bias[gate])
        nc.scalar.activation(gsbs[t][:, gate, :], Gc[:], gate_funcs[gate],
                             bias=bias_imm)

    def mm_gate(t, gate):
        mm_gate_ks(t, gate, 0, KK)

    tanh_cs = {}

    def epilogue_c_phase(t):
        # c = f*c_prev + i*g, tanh(c), and the c store
        q, tq = divmod(t, QUAD)
        c4 = quad_c[q]
        c4_sb, _ = out_tiles[q]
        gsb = gsbs[t]
        i_s, f_s, g_t = gsb[:, 0, :], gsb[:, 1, :], gsb[:, 2, :]
        tmp = epi.tile([P, H], BF16, tag="tmp")
        nc.vector.tensor_mul(tmp[:], i_s, g_t)
        c1 = epi.tile([P, H], BF16, tag="c1")
        if t == NT - 1:
            # last tile: keep the c chain off the slow GPSIMD path so tanh(c)
            # completes while the o-gate matmuls are still running
            nc.vector.tensor_mul(c1[:], f_s, c4[:, tq, :])
        else:
            nc.gpsimd.tensor_mul(c1[:], f_s, c4[:, tq, :])
        nc.vector.tensor_add(c4_sb[:, tq, :], c1[:], tmp[:])
        tanh_c = epi.tile([P, H], BF16, tag="tanh_c")
        nc.scalar.activation(tanh_c[:], c4_sb[:, tq, :], AF.Tanh)
        tanh_cs[t] = tanh_c
        if q == NT // QUAD - 1:  # per-tile store in the last quad (short tail)
            rows = slice(t * P, (t + 1) * P)
            nc.sync.dma_start(out=co_d[rows, :].rearrange("(n p) d -> p n d", p=P),
                              in_=c4_sb[:, tq:tq + 1, :])
        elif tq == QUAD - 1:
            nc.sync.dma_start(out=dram_quad(co_d, q), in_=c4_sb[:])

    def epilogue_tile(t, mv_g, tt, hpres):
        if t not in tanh_cs:
            epilogue_c_phase(t)
        tanh_c = tanh_cs.pop(t)
        o_s = gsbs.pop(t)[:, 3, :]
        h_pre = hpre_pool.tile([P, H], BF16, tag="h_pre")
        nc.vector.tensor_mul(h_pre[:], o_s, tanh_c[:])
        st = stat_pool.tile([P, 6], F32, tag="st")
        nc.vector.bn_stats(out=st[:], in_=h_pre[:])
        nc.vector.bn_aggr(out=mv_g[:, tt, :], in_=st[:])
        hpres.append((t, h_pre))

    def ln_group(g_tiles, mv_g, hpres):
        g_sz = len(g_tiles)
        # ---- batched 1/sqrt(var+eps) via Newton (vector engine only) -------
        mu_v = mv_g[:, 0:g_sz, 0]
        var_v = mv_g[:, 0:g_sz, 1]
        inv_g = grp_pool.tile([P, QUAD], F32, tag="inv_g")
        nms_g = grp_pool.tile([P, QUAD], F32, tag="nms_g")
        v_g = grp_pool.tile([P, QUAD], F32, tag="v_g")
        nc.vector.tensor_scalar_add(v_g[:, 0:g_sz], var_v, LN_EPS)
        y_i = inv_g.bitcast(I32)
        nc.vector.tensor_scalar(y_i[:, 0:g_sz], v_g[:, 0:g_sz].bitcast(I32),
                                1, None, op0=OP.logical_shift_right)
        nc.vector.tensor_sub(y_i[:, 0:g_sz], magic[:, 0:g_sz], y_i[:, 0:g_sz])
        nt1 = grp_pool.tile([P, QUAD], F32, tag="nt1")
        # 1 iteration for the 1-wide tail groups: inv rel-err ~1.7e-3 over
        # 1/16 of rows -> ~4e-4 on h; shortens the critical tail chain.
        iters = 1 if g_sz == 1 else NEWTON_ITERS
        for _ in range(iters):  # Newton: y = y * (1.5 - 0.5 * v * y^2)
            nc.vector.tensor_mul(nt1[:, 0:g_sz], inv_g[:, 0:g_sz], inv_g[:, 0:g_sz])
            nc.vector.tensor_mul(nt1[:, 0:g_sz], nt1[:, 0:g_sz], v_g[:, 0:g_sz])
            nc.vector.tensor_scalar(nt1[:, 0:g_sz], nt1[:, 0:g_sz], -0.5, 1.5,
                                    op0=OP.mult, op1=OP.add)
            nc.vector.tensor_mul(inv_g[:, 0:g_sz], inv_g[:, 0:g_sz], nt1[:, 0:g_sz])
        nc.vector.scalar_tensor_tensor(nms_g[:, 0:g_sz], mu_v, -1.0,
                                       inv_g[:, 0:g_sz], op0=OP.mult, op1=OP.mult)

        # ---- normalize (+ optional ln scale/shift) + store ------------------
        for tt, (t, h_pre) in enumerate(hpres):
            q, tq = divmod(t, QUAD)
            c4_sb, h4_sb = out_tiles[q]
            if ln_identity and t == NT - 1:
                # last tile: normalize on DVE right after the Newton chain
                # (no cross-engine hop before the final store)
                nc.vector.tensor_scalar(h4_sb[:, tq, :], h_pre[:],
                                        inv_g[:, tt:tt + 1],
                                        nms_g[:, tt:tt + 1],
                                        op0=OP.mult, op1=OP.add)
                rows = slice(t * P, (t + 1) * P)
                nc.sync.dma_start(
                    out=ho_d[rows, :].rearrange("(n p) d -> p n d", p=P),
                    in_=h4_sb[:, tq:tq + 1, :])
                continue
            if ln_identity:
                nc.scalar.activation(h4_sb[:, tq, :], h_pre[:], AF.Identity,
                                     bias=nms_g[:, tt:tt + 1],
                                     scale=inv_g[:, tt:tt + 1])
            else:
                h_n = epi.tile([P, H], F32, tag="h_n")
                nc.scalar.activation(h_n[:], h_pre[:], AF.Identity,
                                     bias=nms_g[:, tt:tt + 1],
                                     scale=inv_g[:, tt:tt + 1])
                h1 = epi.tile([P, H], F32, tag="h1")
                nc.gpsimd.tensor_mul(h1[:], h_n[:], lnw_b[:])
                nc.gpsimd.tensor_add(h4_sb[:, tq, :], h1[:], lnb_b[:])
            if q == NT // QUAD - 1:
                rows = slice(t * P, (t + 1) * P)
                nc.sync.dma_start(
                    out=ho_d[rows, :].rearrange("(n p) d -> p n d", p=P),
                    in_=h4_sb[:, tq:tq + 1, :])
            elif tq == QUAD - 1:
                nc.sync.dma_start(out=dram_quad(ho_d, q), in_=h4_sb[:])

    # --- main schedule -------------------------------------------------------
    # Quad 0 runs gate-major so the PE chases the 8 streaming W DMAs without
    # stalling; later quads run tile-major.
    group_of_tile = {}
    groups = []
    t0 = 0
    for sz in LN_GROUPS:
        groups.append(list(range(t0, t0 + sz)))
        for t in range(t0, t0 + sz):
            group_of_tile[t] = len(groups) - 1
        t0 += sz
    group_state = {}  # group idx -> (mv_g, hpres)

    def finish_tile(t):
        gi = group_of_tile[t]
        if gi not in group_state:
            mv_g = grp_pool.tile([P, QUAD, 2], F32, name="mv_g", tag="mv_g")
            group_state[gi] = (mv_g, [])
        mv_g, hpres = group_state[gi]
        tt = t - groups[gi][0]
        epilogue_tile(t, mv_g, tt, hpres)
        if t == groups[gi][-1]:
            ln_group(groups[gi], mv_g, hpres)

    # quad 0 (gate-major; gate 0 additionally split by k-phase so the PE can
    # start on the first x / W_i k-halves while the rest is still streaming;
    # small warm-up filler blocks bridge load-arrival boundaries)
    for t in range(QUAD):
        mm_gate_ks(t, 0, 0, 2)
    for t in range(QUAD):
        mm_gate_ks(t, 0, 2, KX)
    warm_mms(N_WARMUP_MM2)
    for t in range(QUAD):
        mm_gate_ks(t, 0, KX, KK)
    for gate in range(1, 4):
        for t in range(QUAD):
            mm_gate(t, gate)
    for t in range(QUAD):
        finish_tile(t)

    # quads 1..3 (tile-major). The last tile runs its gates g,i,f,o with the
    # c/tanh chain emitted before the o matmuls, so after the final matmul
    # only o -> h_pre -> stats -> rsqrt -> normalize -> store remain.
    for t in range(QUAD, NT):
        q, tq = divmod(t, QUAD)
        if tq == 0 and q + 1 < NT // QUAD:
            load_quad_xh(q + 1)
            load_quad_c(q + 1)
        if t == NT - 1:
            for gate in (2, 0, 1):
                mm_gate(t, gate)
            epilogue_c_phase(t)
            mm_gate(t, 3)
        else:
            for gate in range(4):
                mm_gate(t, gate)
        finish_tile(t)


def _build(gate_bias, ln_identity):
    key = ("nc", gate_bias, ln_identity)
    if key in _CACHE:
        return _CACHE[key]
    from contextlib import ExitStack
    import concourse.tile as tile
    from concourse import bacc

    nc = bacc.Bacc("TRN2", target_bir_lowering=False, debug=False)
    with tile.TileContext(nc) as tc:
        with ExitStack() as ctx:
            _emit(nc, tc, ctx, gate_bias, ln_identity)
    nc.compile()
    _CACHE[key] = nc
    return nc


def kernel(x, h_prev, c_prev, W_i, W_h, b, ln_weight, ln_bias):
    from concourse.bass_utils import run_bass_kernel_spmd

    b = np.asarray(b, dtype=np.float32)
    ln_weight = np.asarray(ln_weight, dtype=np.float32)
    ln_bias = np.asarray(ln_bias, dtype=np.float32)

    # Specialize the compiled program to the actual bias / LN parameters when
    # they have the common structure (per-gate-constant bias, identity LN);
    # general fallback paths otherwise.
    bg = b.reshape(4, H)
    if np.all(bg == bg[:, :1]):
        gate_bias = tuple(float(v) for v in bg[:, 0])
    else:
        gate_bias = None
    ln_identity = bool(np.all(ln_weight == 1.0) and np.all(ln_bias == 0.0))

    nc = _build(gate_bias, ln_identity)
    import ml_dtypes
    bf16 = ml_dtypes.bfloat16
    wi_b = np.asarray(W_i, dtype=bf16)
    wh_b = np.asarray(W_h, dtype=bf16)
    x = np.asarray(x)
    h_prev = np.asarray(h_prev)
    in_maps = []
    for c in range(N_CORES):
        rows = slice(c * BS, (c + 1) * BS)
        in_maps.append({
            # per-shard feature-major bf16 staging of the activations
            "x": np.ascontiguousarray(x[rows].T.astype(bf16)),
            "h_prev": np.ascontiguousarray(h_prev[rows].T.astype(bf16)),
            "c_prev": np.ascontiguousarray(c_prev[rows]).astype(bf16),
            "W_i": wi_b,
            "W_h": wh_b,
            "b": b,
            "ln_weight": ln_weight,
            "ln_bias": ln_bias,
        })
    res = run_bass_kernel_spmd(nc, in_maps, list(range(N_CORES)))
    h = np.concatenate([np.asarray(res.results[c]["h_out"], dtype=np.float32)
                        for c in range(N_CORES)], axis=0)
    c_out = np.concatenate([np.asarray(res.results[c]["c_out"], dtype=np.float32)
                            for c in range(N_CORES)], axis=0)
    return h, c_out
